# revision 44
# baseline (speedup 1.0000x reference)
"""BeatPooling segment-mean kernel for 8 Trainium2 NeuronCores.

Reference computation (per batch row):
    s = clip(bounds[:, 0], 0, T-1); e = max(s+1, min(bounds[:, 1], T))
    mean[m] = sum(frame[s_m:e_m]) / (e_m - s_m)
    out = concat([mean, fourier(pos)], -1) @ W + b         # [M, D]

Sharding: data-parallel over B (one batch row per core).

Algorithm (per core).  The whole thing is matmuls; no gpsimd.

  1. The frame row is pre-cast to fp16 on the host (halves the HBM
     stream; rel-err stays ~6e-3 vs the 2e-2 gate) and laid out so each
     SBUF partition receives 4 *consecutive* frame rows = one contiguous
     4 KiB DMA descriptor (vs the 2 KiB descriptors a plain
     frame-per-partition layout forces; the DMA engines are
     per-descriptor-throughput-bound, not HBM-bound).
  2. The whole frame row lands in one persistent 64 KB/partition SBUF
     slab via ~7 large slab DMAs with no destination reuse - per-block
     dma_starts serialize on the DGE ring's completion handshake
     (~0.9 us each) and trickle; large unconditioned slabs keep the 16
     DMA engines at 100% duty (frame resident by ~38 us).
  3. Edge matmuls.  Per 512-frame superblock k: 4 accumulating matmuls
     (one per within-partition sub-row j) with host-built *fp8*
     stationary masks U_j[p, slot] = [4p + j <= off(slot)] (0/1 exact
     in fp8; mixed fp8 x fp16 matmuls work on TRN2).  Slot 0 of each
     superblock is the all-ones column (the block sum); slots 1.. are
     the distinct segment-boundary positions (s-1 / e-1) falling in
     that superblock.  PSUM result pp[slot, d] holds every within-block
     prefix the output needs; evicted to SBUF as fp16 (pvall), one
     clean [128, 512] tile per superblock.
  4. Combine matmuls.  segT[d, m] = sum_t pvall_t^T . G_t accumulated
     in PSUM over the 16 slot-tiles.  G_t [128 slots, 512 m] (fp8,
     exact +-1/0) carries +1 at each segment's e-boundary slot, -1 at
     the s-boundary slot, and +1 on the slot-0 rows of every
     fully-spanned superblock.  Each tile's combine is issued 3
     superblocks after its edge so the PE never joins on the eviction
     latency - it runs edge and combine back-to-back at full issue rate
     (~215 ns per 512-row matmul).
  5. Projection: 4 d-chunk matmuls with fp16 W1 plus a 5th 64-deep
     chunk [count*ff^T; count] @ [W2; b] that adds the count-scaled
     fourier/bias term inside the same PSUM accumulation; a final
     per-partition 1/count scale (vector/ACT alternating) lands fp16
     output, DMA'd on both rings; the host upcasts to f32.
"""

import math

import numpy as np

import concourse.bacc as bacc
import concourse.mybir as mybir
from concourse import bass_utils
from concourse.tile import TileContext

B, T, D, M = 8, 8192, 512, 512
POS_DIM = 32
P = 128
N_CORES = 8
S = 128                # slots per superblock (= one combine tile)
DC = D // P            # 4 d-chunks
MC = M // P            # 4 m-chunks

F32 = mybir.dt.float32
F16 = mybir.dt.float16
F8 = mybir.dt.float8e4

_CACHED_NC = {}


def _build_nc(SB):
    JS = SB // P           # consecutive frame rows per partition
    NSB = T // SB          # superblocks == combine tiles
    NT = NSB
    NIDX = NSB * JS * S    # mask columns
    NAUX = 1032            # ffcnt | w2pack | recip (f32 as 2xf16)

    # Built as a single-core program: there is no cross-core
    # communication (pure data parallelism), and the multi-device build
    # emits all-core start/end barrier semaphores that cost ~6-8 us.
    nc = bacc.Bacc("TRN2", target_bir_lowering=False, debug=False,
                   num_devices=1)

    frame = nc.dram_tensor("frame", [T, D], F16, kind="ExternalInput")
    us_in = nc.dram_tensor("uslots", [P, NIDX], F8, kind="ExternalInput")
    g_in = nc.dram_tensor("gmat", [P, NT * M], F8, kind="ExternalInput")
    w1_in = nc.dram_tensor("w1p", [P, DC * D], F16, kind="ExternalInput")
    aux_in = nc.dram_tensor("aux", [P, NAUX], F16, kind="ExternalInput")
    out = nc.dram_tensor("out", [M, D], F16, kind="ExternalOutput")

    mult = mybir.AluOpType.mult

    with TileContext(nc, num_cores=1) as tc:
        with (
            tc.tile_pool(name="const", bufs=1) as const,
            tc.tile_pool(name="staging", bufs=1) as staging,
            tc.tile_pool(name="psum", bufs=4, space="PSUM") as psum,
            tc.tile_pool(name="pacc", bufs=1, space="PSUM") as pacc,
        ):
            # ---- long-lived tiles -------------------------------------
            uslots = const.tile([P, NIDX], F8, name="uslots")
            gmat = const.tile([P, NT * M], F8, name="gmat")
            pvall = const.tile([P, NT * D], F16, name="pvall")
            w1t = const.tile([P, DC * D], F16, name="w1t")
            aux = const.tile([P, NAUX], F16, name="aux")
            segsb = const.tile([P, DC * M], F16, name="segsb")
            outall = const.tile([P, MC * D], F16, name="outall")

            # All aux tensors are issued up-front; with the DMA engines
            # saturated early on, only the QUEUE ORDER matters, so they
            # ride interleaved in consumption order: masks for the first
            # superblocks first, then alternating uslots/gmat quarters.
            UQ = NIDX // 4
            GQ = NT * M // 4
            US0 = JS * S
            nc.scalar.dma_start(uslots[:, 0:US0], us_in.ap()[:, 0:US0])
            nc.scalar.dma_start(uslots[:, US0:UQ], us_in.ap()[:, US0:UQ])
            nc.scalar.dma_start(uslots[:, UQ:2 * UQ],
                                us_in.ap()[:, UQ:2 * UQ])
            nc.scalar.dma_start(gmat[:, 0:GQ], g_in.ap()[:, 0:GQ])
            nc.scalar.dma_start(uslots[:, 2 * UQ:3 * UQ],
                                us_in.ap()[:, 2 * UQ:3 * UQ])
            nc.scalar.dma_start(gmat[:, GQ:2 * GQ], g_in.ap()[:, GQ:2 * GQ])
            nc.scalar.dma_start(uslots[:, 3 * UQ:], us_in.ap()[:, 3 * UQ:])
            nc.scalar.dma_start(gmat[:, 2 * GQ:3 * GQ],
                                g_in.ap()[:, 2 * GQ:3 * GQ])
            nc.scalar.dma_start(gmat[:, 3 * GQ:], g_in.ap()[:, 3 * GQ:])
            # w1t/aux are tail-only consumers - they ride last so the
            # early engine bandwidth all goes to the frame slabs
            nc.scalar.dma_start(w1t[:], w1_in.ap())
            nc.scalar.dma_start(aux[:], aux_in.ap())

            # segT accumulators, one per d-chunk, live across the stream
            po = [pacc.tile([P, M], F32, name=f"po_{c}", tag=f"po{c}")
                  for c in range(DC)]

            # One persistent staging slab holds the whole frame row; the
            # stream rides a handful of large slab DMAs with no
            # destination reuse, so the sync queue never waits between
            # transfers (per-superblock dma_starts serialized on the DGE
            # ring's completion handshake and trickled at PE pace).
            stall = staging.tile([P, NSB * JS * D], F16, name="stall")
            stv = stall[:].rearrange("p (k x) -> p k x", k=NSB)
            frame_s = frame.ap().rearrange("(k p j) d -> p k (j d)",
                                           p=P, j=JS)
            h = JS * D // 2
            nc.sync.dma_start(stv[:, 0:1][:, :, 0:h],
                              frame_s[:, 0:1][:, :, 0:h])
            nc.sync.dma_start(stv[:, 0:1][:, :, h:],
                              frame_s[:, 0:1][:, :, h:])
            nc.sync.dma_start(stv[:, 1:2], frame_s[:, 1:2])
            nc.sync.dma_start(stv[:, 2:4], frame_s[:, 2:4])
            for g0 in range(4, NSB, 4):
                nc.sync.dma_start(stv[:, g0:g0 + 4], frame_s[:, g0:g0 + 4])

            # ---- stream compute --------------------------------------
            # The combine for tile t is issued after edge t+2, giving
            # tile t's PSUM->SBUF eviction two superblocks (~3 us) of
            # slack - the PE then runs edge and combine back-to-back with
            # no semaphore joins, stays continuously busy, and ramps to
            # its full clock.
            def combine(t, stop):
                for c in range(DC):
                    nc.tensor.matmul(
                        po[c][:],
                        lhsT=pvall[:, t * D + c * P:t * D + (c + 1) * P],
                        rhs=gmat[:, t * M:(t + 1) * M],
                        start=(t == 0), stop=stop,
                    )

            for k in range(NSB):
                pp = psum.tile([S, D], F32, name=f"pp_{k}", tag="ps")
                for jj in range(JS):
                    nc.tensor.matmul(
                        pp[:],
                        lhsT=uslots[:, (k * JS + jj) * S:
                                    (k * JS + jj + 1) * S],
                        rhs=stall[:, (k * JS + jj) * D:
                                  (k * JS + jj + 1) * D],
                        start=(jj == 0), stop=(jj == JS - 1),
                    )
                if k % 2 == 0:
                    nc.vector.tensor_scalar_add(
                        out=pvall[:, k * D:(k + 1) * D], in0=pp[:],
                        scalar1=0.0)
                else:
                    nc.scalar.copy(pvall[:, k * D:(k + 1) * D], pp[:])

                if k >= 3:
                    combine(k - 3, stop=False)

            # ---- tail: last combines, evict, project, scale, store ----
            # Per d-chunk: close the po[c] accumulation, evict it, and
            # let its projection matmuls run while the next chunk
            # combines - keeps the PE busy through the whole tail.
            recip_v = aux[:, 1024:1032].bitcast(F32)     # [128, 4]
            po2 = [psum.tile([P, D], F32, name=f"po2_{mt}", tag="ps")
                   for mt in range(MC)]
            for c in range(DC):
                for t in (NT - 3, NT - 2, NT - 1):
                    nc.tensor.matmul(
                        po[c][:],
                        lhsT=pvall[:, t * D + c * P:t * D + (c + 1) * P],
                        rhs=gmat[:, t * M:(t + 1) * M],
                        start=False, stop=(t == NT - 1),
                    )
                if c % 2 == 0:
                    nc.vector.tensor_scalar_add(
                        out=segsb[:, c * M:(c + 1) * M], in0=po[c][:],
                        scalar1=0.0)
                else:
                    nc.scalar.copy(segsb[:, c * M:(c + 1) * M], po[c][:])
                for mt in range(MC):
                    nc.tensor.matmul(
                        po2[mt][:],
                        lhsT=segsb[:, c * M + mt * P:c * M + (mt + 1) * P],
                        rhs=w1t[:, c * D:(c + 1) * D],
                        start=(c == 0), stop=False,
                    )
            outv = out.ap().rearrange("(mt p) d -> p mt d", p=P)
            oall = outall[:].rearrange("p (mt d) -> p mt d", mt=MC)
            for mt in range(MC):
                nc.tensor.matmul(
                    po2[mt][:],
                    lhsT=aux[0:64, mt * P:(mt + 1) * P],
                    rhs=aux[0:64, 512:1024],
                    start=False, stop=True,
                )
                # fold 1/count back in (gmat is exact-fp8 +-1/0, so the
                # per-m scale rides here as a per-partition scalar);
                # alternate engines so the four scales pipeline
                if mt % 2 == 0:
                    nc.vector.tensor_scalar(
                        out=outall[:, mt * D:(mt + 1) * D],
                        in0=po2[mt][:],
                        scalar1=recip_v[:, mt:mt + 1],
                        scalar2=None,
                        op0=mult,
                    )
                else:
                    nc.scalar.mul(outall[:, mt * D:(mt + 1) * D],
                                  po2[mt][:], recip_v[:, mt:mt + 1])
                ring = nc.sync if mt % 2 == 0 else nc.scalar
                ring.dma_start(outv[:, mt:mt + 1], oall[:, mt:mt + 1])

    nc.compile()
    return nc


def _fourier_features(pos, dim):
    half = dim // 2
    freqs = np.exp(np.linspace(0.0, math.log(1000.0), half))
    ang = pos[..., None] * freqs
    return np.concatenate([np.sin(ang), np.cos(ang)], axis=-1)


def _host_prep(frame_emb, beat_bounds, W, b, SB):
    JS = SB // P
    NSB = T // SB
    NT = NSB
    NIDX = NSB * JS * S

    s_all = np.clip(beat_bounds[:, :, 0], 0, T - 1).astype(np.int64)
    e_all = np.maximum(
        s_all + 1, np.minimum(beat_bounds[:, :, 1], T)).astype(np.int64)
    recip_all = (1.0 / (e_all - s_all)).astype(np.float32)

    pos = np.clip(np.arange(M, dtype=np.float64) / max(1, M - 1), 0.0, 1.0)
    ff = _fourier_features(pos, POS_DIM)                  # [M, 32]

    w1p = np.ascontiguousarray(
        W[:D, :].astype(np.float16).reshape(DC, P, D)
        .transpose(1, 0, 2).reshape(P, DC * D))
    f8 = mybir.dt.np(mybir.dt.float8e4)

    in_maps = []
    for i in range(B):
        s, e, recip = s_all[i], e_all[i], recip_all[i]
        count = (e - s).astype(np.float32)

        # aux: count-scaled fourier columns (so the bias term rides the
        # projection accumulation), W2/b rows, and the 1/count scales
        aux = np.zeros((P, 1032), dtype=np.float16)
        aux[0:POS_DIM, 0:M] = (ff.T * count[None, :]).astype(np.float16)
        aux[POS_DIM, 0:M] = count.astype(np.float16)
        aux[0:POS_DIM, 512:512 + D] = W[D:D + POS_DIM, :].astype(np.float16)
        aux[POS_DIM, 512:512 + D] = b.astype(np.float16)
        aux[:, 1024:1032] = np.ascontiguousarray(
            recip.reshape(MC, P).T, dtype=np.float32).view(np.float16)
        allpos = np.concatenate([(s - 1)[s > 0], e - 1])
        idxv = np.full(NIDX, -1.0, dtype=np.float32)
        slotmap = {}
        for k in range(NSB):
            offs = np.unique(allpos[(allpos // SB) == k] % SB)
            if len(offs) > S - 1:
                raise OverflowError(
                    f"superblock {k}: {len(offs)} boundaries > {S - 1}")
            base = k * JS * S
            for jj in range(JS):
                idxv[base + jj * S] = (SB - 1 - jj) // JS   # sum slot
                idxv[base + jj * S + 1:base + jj * S + 1 + len(offs)] = (
                    np.where(offs >= jj, (offs - jj) // JS, -1.0))
            for j, o in enumerate(offs):
                slotmap[(k, int(o))] = j + 1

        # stationary masks: uslots[p, c] = (idxv[c] >= p)
        us = (idxv[None, :] >= np.arange(P, dtype=np.float32)[:, None])

        # G[slot, m]: +1 at e-boundary slot, -1 at s-boundary slot,
        # +1 on the sum slots of fully-spanned superblocks (exact in
        # fp8; the 1/count scale is applied after projection)
        gm = np.zeros((NSB * S, M), dtype=np.float32)
        for m in range(M):
            pe = int(e[m]) - 1
            ke = pe // SB
            gm[ke * S + slotmap[(ke, pe % SB)], m] += 1.0
            ks = 0
            if s[m] > 0:
                ps = int(s[m]) - 1
                ks = ps // SB
                gm[ks * S + slotmap[(ks, ps % SB)], m] -= 1.0
            for k in range(ks, ke):
                gm[k * S, m] += 1.0
        gmat = np.ascontiguousarray(
            gm.reshape(NT, P, M).transpose(1, 0, 2)
            .reshape(P, NT * M)).astype(f8)

        in_maps.append({
            "frame": np.ascontiguousarray(frame_emb[i], dtype=np.float16),
            "uslots": us.astype(f8),
            "gmat": gmat,
            "w1p": w1p,
            "aux": aux,
        })
    return in_maps


def get_nc(SB=512):
    if SB not in _CACHED_NC:
        _CACHED_NC[SB] = _build_nc(SB)
    return _CACHED_NC[SB]


def kernel(frame_emb, beat_bounds, W, b, _trace=False):
    frame_emb = np.asarray(frame_emb)
    beat_bounds = np.asarray(beat_bounds)
    W = np.asarray(W)
    b = np.asarray(b)
    in_maps = None
    for SB in (512, 256, 128):
        try:
            in_maps = _host_prep(frame_emb, beat_bounds, W, b, SB)
            break
        except OverflowError:
            continue
    if in_maps is None:
        raise RuntimeError("too many segment boundaries per superblock")
    nc = get_nc(SB)
    res = bass_utils.run_bass_kernel_spmd(
        nc, in_maps, core_ids=list(range(N_CORES)), trace=_trace)
    out = np.stack([res.results[i]["out"] for i in range(B)],
                   axis=0).astype(np.float32)
    if _trace:
        kernel.last_results = res
    return out
